# revision 12
# baseline (speedup 1.0000x reference)
"""Trainium2 Bass kernel for nn_ConcatLayer_37589553774933 (topk_masking).

Per-row computation on [N, 9] f32 (N = 8388608): three groups of 3
(up/none/down); per group a strict-argmax code in {-1,0,1}; a scalar
decision chain (calc/sign/idx); masking; probe-argmax group selection;
output [N, 3].

Sharding: trivially data-parallel — rows are split evenly across the 8
NeuronCores; each core runs the same SPMD kernel on its shard.

Layout per core: rows are tiled as [T, 128, C] (partition-major blocks of
C consecutive rows), so every DMA is a contiguous 9*C-float run per
partition. All per-row math runs along the free dimension with strided
views over the packed [C,9] per-partition layout; group-level quantities
are [C,3] packed planes so one instruction covers all three groups.
Selections (idx-gather, group choice) use copy_predicated chains with
uint8 masks broadcast via 0-step access patterns.
"""

import os

import numpy as np

import concourse.bass as bass
import concourse.mybir as mybir
from concourse.tile import TileContext

F32 = mybir.dt.float32
U8 = mybir.dt.uint8
OP = mybir.AluOpType

N_TOTAL = 8388608
N_CORES = 8
R_CORE = N_TOTAL // N_CORES  # 1048576 rows per core
P = 128


def _copy_pred(eng, out, mask, data):
    """copy_predicated with un-optimized APs so all operands keep the same
    3D shape (the stock wrapper's AP optimizer collapses contiguous operands
    to 2D while a 0-step broadcast mask stays 3D, which CoreSim rejects)."""
    return eng.add_instruction(
        mybir.InstCopyPredicated(
            name=f"I-{eng.bass.next_id()}",
            ins=[eng.lower_ap(mask, opt=False), eng.lower_ap(data, opt=False)],
            outs=[eng.lower_ap(out, opt=False)],
        )
    )


def split_multi_waits(nc, max_waits: int = 1):
    """The walrus build in this container rejects instructions carrying more
    than one sync wait (CTRL NO_STRUCT 'Too many sync wait commands').
    Tile's kernel-tail drain accumulates one wait per engine/queue, so
    redistribute excess waits onto same-engine NoOps placed just before."""
    n_split = 0
    for f in nc.m.functions:
        for b in f.blocks:
            new_insts = []
            for ins in b.instructions:
                si = getattr(ins, "sync_info", None)
                if si and si.on_wait and len(si.on_wait) > max_waits:
                    waits = list(si.on_wait)
                    head, tail = waits[:-max_waits], waits[-max_waits:]
                    for k in range(0, len(head), max_waits):
                        chunk = head[k : k + max_waits]
                        nop = mybir.InstNoOp(
                            name=f"{ins.name}_waitsplit{k}",
                            ins=[],
                            outs=[],
                            sync_info=mybir.SyncInfo(
                                on_wait=list(chunk), on_update=[]
                            ),
                        )
                        nop.engine = ins.engine
                        new_insts.append(nop)
                    si.on_wait = tail
                    n_split += 1
                new_insts.append(ins)
            b.instructions = new_insts
    return n_split


def build_nc(R: int, C: int, use_gpsimd: bool = False, use_act: bool = True):
    """Build the SPMD Bass module for one core processing R rows, C rows per
    partition per tile."""
    T = R // (P * C)
    assert R == T * P * C, (R, C)
    nc = bass.Bass("TRN2", debug=False)
    x = nc.declare_dram_parameter("x", [R, 9], F32, isOutput=False)
    o = nc.declare_dram_parameter("o", [R, 3], F32, isOutput=True)
    xt = x[:].rearrange("(t p c) j -> t p (c j)", t=T, p=P, c=C)
    ot = o[:].rearrange("(t p c) j -> t p (c j)", t=T, p=P, c=C)

    dve = nc.vector
    gp = nc.gpsimd if use_gpsimd else nc.vector

    with TileContext(nc) as tc:
        with tc.tile_pool(name="pool", bufs=2) as pool:
            for t in range(T):
                tin = pool.tile([P, 9 * C], F32, name="tin")
                nc.sync.dma_start(tin[:], xt[t])
                tv = tin[:].rearrange("p (c g j) -> p c g j", c=C, g=3, j=3)
                V = [tv[:, :, :, j] for j in range(3)]  # [P, C, 3] strided

                # --- group strict-argmax codes: M = (v0>max(v1,v2)) - (v2>max(v0,v1))
                B = pool.tile([P, 3 * C], F32, name="B", bufs=1)
                Bv = B[:].rearrange("p (c g) -> p c g", g=3)
                dve.tensor_tensor(Bv, V[1], V[2], op=OP.max)
                B2 = pool.tile([P, 3 * C], F32, name="B2", bufs=1)
                B2v = B2[:].rearrange("p (c g) -> p c g", g=3)
                dve.tensor_tensor(B2v, V[0], V[1], op=OP.max)
                TP = pool.tile([P, 3 * C], F32, name="TP", bufs=1)
                TPv = TP[:].rearrange("p (c g) -> p c g", g=3)
                dve.tensor_tensor(TPv, V[0], Bv, op=OP.is_gt)
                TM = pool.tile([P, 3 * C], F32, name="TM", bufs=1)
                TMv = TM[:].rearrange("p (c g) -> p c g", g=3)
                dve.tensor_tensor(TMv, V[2], B2v, op=OP.is_gt)
                M = pool.tile([P, 3 * C], F32, name="M", bufs=1)
                gp.tensor_tensor(M[:], TP[:], TM[:], op=OP.subtract)
                Mv = M[:].rearrange("p (c g) -> p c g", g=3)

                # --- scalar chain: calc = mn^2 * (mu + md + mn)
                S2 = pool.tile([P, C], F32, name="S2")
                gp.tensor_tensor(S2[:], Mv[:, :, 0], Mv[:, :, 2], op=OP.add)
                S3 = pool.tile([P, C], F32, name="S3")
                gp.tensor_tensor(S3[:], S2[:], Mv[:, :, 1], op=OP.add)
                T1 = pool.tile([P, C], F32, name="T1")
                gp.tensor_tensor(T1[:], Mv[:, :, 1], S3[:], op=OP.mult)
                CALC = pool.tile([P, C], F32, name="CALC")
                gp.tensor_tensor(CALC[:], Mv[:, :, 1], T1[:], op=OP.mult)

                # sgn = clamp(calc, -1, 1); e0 = calc==1; e1 = calc==0
                SGA = pool.tile([P, C], F32, name="SGA")
                dve.tensor_scalar_max(SGA[:], CALC[:], -1.0)
                SGN = pool.tile([P, C], F32, name="SGN")
                dve.tensor_scalar_min(SGN[:], SGA[:], 1.0)
                E0 = pool.tile([P, C], U8, name="E0")
                dve.tensor_scalar(E0[:], CALC[:], 1.0, None, op0=OP.is_equal)
                E1 = pool.tile([P, C], U8, name="E1")
                dve.tensor_scalar(E1[:], CALC[:], 0.0, None, op0=OP.is_equal)

                # --- masks: MASK[g] = (M[g] == sgn)
                MASK = pool.tile([P, 3 * C], F32, name="MASK", bufs=1)
                MASKv = MASK[:].rearrange("p (c g) -> p c g", g=3)
                sgnb = SGN[:].broadcast_to((P, C, 3))
                dve.tensor_tensor(MASKv, Mv, sgnb, op=OP.is_equal)

                # --- masked groups (j-indexed planes, g packed inner)
                GM = []
                for j in range(3):
                    gm = pool.tile([P, 3 * C], F32, name=f"GM{j}")
                    gmv = gm[:].rearrange("p (c g) -> p c g", g=3)
                    eng = gp if j == 2 else dve
                    eng.tensor_tensor(gmv, V[j], MASKv, op=OP.mult)
                    GM.append(gm)
                GMv = [gm[:].rearrange("p (c g) -> p c g", g=3) for gm in GM]

                # --- probe: PR[g] = GM_idx[g]; idx: 2 default, 1 if calc==0, 0 if calc==1
                PR = pool.tile([P, 3 * C], F32, name="PR")
                PRv = PR[:].rearrange("p (c g) -> p c g", g=3)
                dve.tensor_copy(PR[:], GM[2][:])
                e1b = E1[:].broadcast_to((P, C, 3))
                e0b = E0[:].broadcast_to((P, C, 3))
                _copy_pred(dve, PRv, e1b, GMv[1])
                _copy_pred(dve, PRv, e0b, GMv[0])

                # --- choice: first-argmax of (pu, pn, pd)
                CN = pool.tile([P, C], U8, name="CN")
                dve.tensor_tensor(CN[:], PRv[:, :, 1], PRv[:, :, 2], op=OP.is_ge)
                MND = pool.tile([P, C], F32, name="MND")
                dve.tensor_tensor(MND[:], PRv[:, :, 1], PRv[:, :, 2], op=OP.max)
                CU = pool.tile([P, C], U8, name="CU")
                dve.tensor_tensor(CU[:], PRv[:, :, 0], MND[:], op=OP.is_ge)

                # --- output select: O[:, j] = GM_j[:, choice]
                OT = pool.tile([P, 3 * C], F32, name="OT")
                Ov = OT[:].rearrange("p (c j) -> p c j", j=3)
                for j in range(3):
                    ocol = Ov[:, :, j]
                    if use_act:
                        nc.scalar.copy(ocol, GMv[j][:, :, 2])
                    else:
                        dve.tensor_copy(ocol, GMv[j][:, :, 2])
                    _copy_pred(dve, ocol, CN[:], GMv[j][:, :, 1])
                    _copy_pred(dve, ocol, CU[:], GMv[j][:, :, 0])

                nc.sync.dma_start(ot[t], OT[:])

    return nc


# --------------------------------------------------------------------------
# Entry point
# --------------------------------------------------------------------------
_BUILT = {}


def _get_nc(R: int, C: int):
    key = (R, C)
    if key not in _BUILT:
        nc = build_nc(R, C)
        split_multi_waits(nc)
        _BUILT[key] = nc
    return _BUILT[key]


KERNEL_C = int(os.environ.get("KERNEL_C", "512"))


def kernel(inputs) -> np.ndarray:
    x = np.ascontiguousarray(np.asarray(inputs, dtype=np.float32))
    n = x.shape[0]
    assert n % N_CORES == 0
    r = n // N_CORES
    nc = _get_nc(r, KERNEL_C)
    shards = x.reshape(N_CORES, r, 9)
    from concourse.bass_utils import run_bass_kernel_spmd

    res = run_bass_kernel_spmd(
        nc, [{"x": shards[i]} for i in range(N_CORES)], core_ids=list(range(N_CORES))
    )
    return np.concatenate([r_["o"] for r_ in res.results], axis=0)


# revision 13
# speedup vs baseline: 1.1659x; 1.1659x over previous
"""Trainium2 Bass kernel for nn_ConcatLayer_37589553774933 (topk_masking).

Per-row computation on [N, 9] f32 (N = 8388608): three groups of 3
(up/none/down); per group a strict-argmax code in {-1,0,1}; a scalar
decision chain (calc/sign/idx); masking; probe-argmax group selection;
output [N, 3].

Sharding: trivially data-parallel — rows are split evenly across the 8
NeuronCores; each core runs the same SPMD kernel on its shard.

Layout per core: rows are tiled as [T, 128, C] (partition-major blocks of
C consecutive rows), so every DMA is a contiguous 9*C-float run per
partition. All per-row math runs along the free dimension with strided
views over the packed [C,9] per-partition layout; group-level quantities
are [C,3] packed planes so one instruction covers all three groups.
Selections (idx-gather, group choice) use copy_predicated chains with
uint8 masks broadcast via 0-step access patterns.
"""

import os

import numpy as np

import concourse.bass as bass
import concourse.mybir as mybir
from concourse.tile import TileContext

F32 = mybir.dt.float32
U8 = mybir.dt.uint8
OP = mybir.AluOpType

N_TOTAL = 8388608
N_CORES = 8
R_CORE = N_TOTAL // N_CORES  # 1048576 rows per core
P = 128


def _copy_pred(eng, out, mask, data):
    """copy_predicated with un-optimized APs so all operands keep the same
    3D shape (the stock wrapper's AP optimizer collapses contiguous operands
    to 2D while a 0-step broadcast mask stays 3D, which CoreSim rejects)."""
    return eng.add_instruction(
        mybir.InstCopyPredicated(
            name=f"I-{eng.bass.next_id()}",
            ins=[eng.lower_ap(mask, opt=False), eng.lower_ap(data, opt=False)],
            outs=[eng.lower_ap(out, opt=False)],
        )
    )


def split_multi_waits(nc, max_waits: int = 1):
    """The walrus build in this container rejects instructions carrying more
    than one sync wait (CTRL NO_STRUCT 'Too many sync wait commands').
    Tile's kernel-tail drain accumulates one wait per engine/queue, so
    redistribute excess waits onto same-engine NoOps placed just before."""
    n_split = 0
    for f in nc.m.functions:
        for b in f.blocks:
            new_insts = []
            for ins in b.instructions:
                si = getattr(ins, "sync_info", None)
                if si and si.on_wait and len(si.on_wait) > max_waits:
                    waits = list(si.on_wait)
                    head, tail = waits[:-max_waits], waits[-max_waits:]
                    for k in range(0, len(head), max_waits):
                        chunk = head[k : k + max_waits]
                        nop = mybir.InstNoOp(
                            name=f"{ins.name}_waitsplit{k}",
                            ins=[],
                            outs=[],
                            sync_info=mybir.SyncInfo(
                                on_wait=list(chunk), on_update=[]
                            ),
                        )
                        nop.engine = ins.engine
                        new_insts.append(nop)
                    si.on_wait = tail
                    n_split += 1
                new_insts.append(ins)
            b.instructions = new_insts
    return n_split


def build_nc(R: int, C: int, use_gpsimd: bool = False, use_act: bool = True):
    """Build the SPMD Bass module for one core processing R rows, C rows per
    partition per tile."""
    T = R // (P * C)
    assert R == T * P * C, (R, C)
    nc = bass.Bass("TRN2", debug=False)
    x = nc.declare_dram_parameter("x", [R, 9], F32, isOutput=False)
    o = nc.declare_dram_parameter("o", [R, 3], F32, isOutput=True)
    xt = x[:].rearrange("(t p c) j -> t p (c j)", t=T, p=P, c=C)
    ot = o[:].rearrange("(t p c) j -> t p (c j)", t=T, p=P, c=C)

    dve = nc.vector
    gp = nc.gpsimd if use_gpsimd else nc.vector

    with TileContext(nc) as tc:
        with tc.tile_pool(name="pool", bufs=2) as pool:
            for t in range(T):
                tin = pool.tile([P, 9 * C], F32, name="tin")
                nc.sync.dma_start(tin[:], xt[t])
                tv = tin[:].rearrange("p (c g j) -> p c g j", c=C, g=3, j=3)
                V = [tv[:, :, :, j] for j in range(3)]  # [P, C, 3] strided

                # --- group strict-argmax codes: M = (v0>max(v1,v2)) - (v2>max(v0,v1))
                B = pool.tile([P, 3 * C], F32, name="B", bufs=1)
                Bv = B[:].rearrange("p (c g) -> p c g", g=3)
                dve.tensor_tensor(Bv, V[1], V[2], op=OP.max)
                B2 = pool.tile([P, 3 * C], F32, name="B2", bufs=1)
                B2v = B2[:].rearrange("p (c g) -> p c g", g=3)
                dve.tensor_tensor(B2v, V[0], V[1], op=OP.max)
                TP = pool.tile([P, 3 * C], F32, name="TP", bufs=1)
                TPv = TP[:].rearrange("p (c g) -> p c g", g=3)
                dve.tensor_tensor(TPv, V[0], Bv, op=OP.is_gt)
                TM = pool.tile([P, 3 * C], F32, name="TM", bufs=1)
                TMv = TM[:].rearrange("p (c g) -> p c g", g=3)
                dve.tensor_tensor(TMv, V[2], B2v, op=OP.is_gt)
                M = pool.tile([P, 3 * C], F32, name="M", bufs=1)
                gp.tensor_tensor(M[:], TP[:], TM[:], op=OP.subtract)
                Mv = M[:].rearrange("p (c g) -> p c g", g=3)

                # --- scalar chain: calc = mn^2 * (mu + md + mn)
                S2 = pool.tile([P, C], F32, name="S2")
                gp.tensor_tensor(S2[:], Mv[:, :, 0], Mv[:, :, 2], op=OP.add)
                S3 = pool.tile([P, C], F32, name="S3")
                gp.tensor_tensor(S3[:], S2[:], Mv[:, :, 1], op=OP.add)
                T1 = pool.tile([P, C], F32, name="T1")
                gp.tensor_tensor(T1[:], Mv[:, :, 1], S3[:], op=OP.mult)
                CALC = pool.tile([P, C], F32, name="CALC")
                gp.tensor_tensor(CALC[:], Mv[:, :, 1], T1[:], op=OP.mult)

                # sgn = clamp(calc, -1, 1); e0 = calc==1; e1 = calc==0
                SGA = pool.tile([P, C], F32, name="SGA")
                dve.tensor_scalar_max(SGA[:], CALC[:], -1.0)
                SGN = pool.tile([P, C], F32, name="SGN")
                dve.tensor_scalar_min(SGN[:], SGA[:], 1.0)
                E0 = pool.tile([P, C], U8, name="E0")
                dve.tensor_scalar(E0[:], CALC[:], 1.0, None, op0=OP.is_equal)
                E1 = pool.tile([P, C], U8, name="E1")
                dve.tensor_scalar(E1[:], CALC[:], 0.0, None, op0=OP.is_equal)

                # --- masks: MASK[g] = (M[g] == sgn)
                MASK = pool.tile([P, 3 * C], F32, name="MASK", bufs=1)
                MASKv = MASK[:].rearrange("p (c g) -> p c g", g=3)
                sgnb = SGN[:].broadcast_to((P, C, 3))
                dve.tensor_tensor(MASKv, Mv, sgnb, op=OP.is_equal)

                # --- masked groups (j-indexed planes, g packed inner)
                GM = []
                for j in range(3):
                    gm = pool.tile([P, 3 * C], F32, name=f"GM{j}")
                    gmv = gm[:].rearrange("p (c g) -> p c g", g=3)
                    eng = gp if j == 2 else dve
                    eng.tensor_tensor(gmv, V[j], MASKv, op=OP.mult)
                    GM.append(gm)
                GMv = [gm[:].rearrange("p (c g) -> p c g", g=3) for gm in GM]

                # --- probe: PR[g] = GM_idx[g]; idx: 2 default, 1 if calc==0, 0 if calc==1
                PR = pool.tile([P, 3 * C], F32, name="PR")
                PRv = PR[:].rearrange("p (c g) -> p c g", g=3)
                dve.tensor_copy(PR[:], GM[2][:])
                e1b = E1[:].broadcast_to((P, C, 3))
                e0b = E0[:].broadcast_to((P, C, 3))
                _copy_pred(dve, PRv, e1b, GMv[1])
                _copy_pred(dve, PRv, e0b, GMv[0])

                # --- choice: first-argmax of (pu, pn, pd)
                CN = pool.tile([P, C], U8, name="CN")
                dve.tensor_tensor(CN[:], PRv[:, :, 1], PRv[:, :, 2], op=OP.is_ge)
                MND = pool.tile([P, C], F32, name="MND")
                dve.tensor_tensor(MND[:], PRv[:, :, 1], PRv[:, :, 2], op=OP.max)
                CU = pool.tile([P, C], U8, name="CU")
                dve.tensor_tensor(CU[:], PRv[:, :, 0], MND[:], op=OP.is_ge)

                # --- output select: O[:, j] = GM_j[:, choice]
                OT = pool.tile([P, 3 * C], F32, name="OT")
                Ov = OT[:].rearrange("p (c j) -> p c j", j=3)
                for j in range(3):
                    ocol = Ov[:, :, j]
                    if use_act:
                        nc.scalar.copy(ocol, GMv[j][:, :, 2])
                    else:
                        dve.tensor_copy(ocol, GMv[j][:, :, 2])
                    _copy_pred(dve, ocol, CN[:], GMv[j][:, :, 1])
                    _copy_pred(dve, ocol, CU[:], GMv[j][:, :, 0])

                nc.sync.dma_start(ot[t], OT[:])

    return nc


# --------------------------------------------------------------------------
# Entry point
# --------------------------------------------------------------------------
_BUILT = {}


def _get_nc(R: int, C: int):
    key = (R, C)
    if key not in _BUILT:
        nc = build_nc(
            R, C, use_gpsimd=os.environ.get("USE_GP", "0") == "1"
        )
        split_multi_waits(nc)
        _BUILT[key] = nc
    return _BUILT[key]


KERNEL_C = int(os.environ.get("KERNEL_C", "512"))


def kernel(inputs) -> np.ndarray:
    x = np.ascontiguousarray(np.asarray(inputs, dtype=np.float32))
    n = x.shape[0]
    assert n % N_CORES == 0
    r = n // N_CORES
    nc = _get_nc(r, KERNEL_C)
    shards = x.reshape(N_CORES, r, 9)
    from concourse.bass_utils import run_bass_kernel_spmd

    res = run_bass_kernel_spmd(
        nc, [{"x": shards[i]} for i in range(N_CORES)], core_ids=list(range(N_CORES))
    )
    return np.concatenate([r_["o"] for r_ in res.results], axis=0)


# revision 14
# speedup vs baseline: 1.4875x; 1.2759x over previous
"""Trainium2 Bass kernel for nn_ConcatLayer_37589553774933 (topk_masking).

Per-row computation on [N, 9] f32 (N = 8388608): three groups of 3
(up/none/down); per group a strict-argmax code in {-1,0,1}; a scalar
decision chain (calc/sign/idx); masking; probe-argmax group selection;
output [N, 3].

Sharding: trivially data-parallel — rows are split evenly across the 8
NeuronCores; each core runs the same SPMD kernel on its shard.

Layout per core: rows are tiled as [T, 128, C] (partition-major blocks of
C consecutive rows), so every DMA is a contiguous 9*C-float run per
partition. All per-row math runs along the free dimension with strided
views over the packed [C,9] per-partition layout; group-level quantities
are [C,3] packed planes so one instruction covers all three groups.
Selections (idx-gather, group choice) use copy_predicated chains with
uint8 masks broadcast via 0-step access patterns.
"""

import os

import numpy as np

import concourse.bass as bass
import concourse.mybir as mybir
from concourse.tile import TileContext

F32 = mybir.dt.float32
U8 = mybir.dt.uint8
OP = mybir.AluOpType

N_TOTAL = 8388608
N_CORES = 8
R_CORE = N_TOTAL // N_CORES  # 1048576 rows per core
P = 128


def _copy_pred(eng, out, mask, data):
    """copy_predicated with un-optimized APs so all operands keep the same
    3D shape (the stock wrapper's AP optimizer collapses contiguous operands
    to 2D while a 0-step broadcast mask stays 3D, which CoreSim rejects)."""
    return eng.add_instruction(
        mybir.InstCopyPredicated(
            name=f"I-{eng.bass.next_id()}",
            ins=[eng.lower_ap(mask, opt=False), eng.lower_ap(data, opt=False)],
            outs=[eng.lower_ap(out, opt=False)],
        )
    )


def split_multi_waits(nc, max_waits: int = 1):
    """The walrus build in this container rejects instructions carrying more
    than one sync wait (CTRL NO_STRUCT 'Too many sync wait commands').
    Tile's kernel-tail drain accumulates one wait per engine/queue, so
    redistribute excess waits onto same-engine NoOps placed just before."""
    n_split = 0
    for f in nc.m.functions:
        for b in f.blocks:
            new_insts = []
            for ins in b.instructions:
                si = getattr(ins, "sync_info", None)
                if si and si.on_wait and len(si.on_wait) > max_waits:
                    waits = list(si.on_wait)
                    head, tail = waits[:-max_waits], waits[-max_waits:]
                    for k in range(0, len(head), max_waits):
                        chunk = head[k : k + max_waits]
                        nop = mybir.InstNoOp(
                            name=f"{ins.name}_waitsplit{k}",
                            ins=[],
                            outs=[],
                            sync_info=mybir.SyncInfo(
                                on_wait=list(chunk), on_update=[]
                            ),
                        )
                        nop.engine = ins.engine
                        new_insts.append(nop)
                    si.on_wait = tail
                    n_split += 1
                new_insts.append(ins)
            b.instructions = new_insts
    return n_split


def build_nc(R: int, C: int, use_gpsimd: bool = True, use_act: bool = True):
    """Build the SPMD Bass module for one core processing R rows, C rows per
    partition per tile."""
    T = R // (P * C)
    assert R == T * P * C, (R, C)
    nc = bass.Bass("TRN2", debug=False)
    x = nc.declare_dram_parameter("x", [R, 9], F32, isOutput=False)
    o = nc.declare_dram_parameter("o", [R, 3], F32, isOutput=True)
    xt = x[:].rearrange("(t p c) j -> t p (c j)", t=T, p=P, c=C)
    ot = o[:].rearrange("(t p c) j -> t p (c j)", t=T, p=P, c=C)

    dve = nc.vector
    gp = nc.gpsimd if use_gpsimd else nc.vector

    with TileContext(nc) as tc:
        with tc.tile_pool(name="pool", bufs=2) as pool:
            for t in range(T):
                tin = pool.tile([P, 9 * C], F32, name="tin")
                nc.sync.dma_start(tin[:], xt[t])
                tv = tin[:].rearrange("p (c g j) -> p c g j", c=C, g=3, j=3)
                V = [tv[:, :, :, j] for j in range(3)]  # [P, C, 3] strided

                # --- group strict-argmax codes: M = (v0>max(v1,v2)) - (v2>max(v0,v1))
                B = pool.tile([P, 3 * C], F32, name="B", bufs=1)
                Bv = B[:].rearrange("p (c g) -> p c g", g=3)
                dve.tensor_tensor(Bv, V[1], V[2], op=OP.max)
                B2 = pool.tile([P, 3 * C], F32, name="B2", bufs=1)
                B2v = B2[:].rearrange("p (c g) -> p c g", g=3)
                dve.tensor_tensor(B2v, V[0], V[1], op=OP.max)
                TP = pool.tile([P, 3 * C], F32, name="TP", bufs=1)
                TPv = TP[:].rearrange("p (c g) -> p c g", g=3)
                dve.tensor_tensor(TPv, V[0], Bv, op=OP.is_gt)
                TM = pool.tile([P, 3 * C], F32, name="TM", bufs=1)
                TMv = TM[:].rearrange("p (c g) -> p c g", g=3)
                dve.tensor_tensor(TMv, V[2], B2v, op=OP.is_gt)
                M = pool.tile([P, 3 * C], F32, name="M", bufs=1)
                gp.tensor_tensor(M[:], TP[:], TM[:], op=OP.subtract)
                Mv = M[:].rearrange("p (c g) -> p c g", g=3)

                # --- scalar chain: calc = mn^2 * (mu + md + mn)
                S2 = pool.tile([P, C], F32, name="S2")
                gp.tensor_tensor(S2[:], Mv[:, :, 0], Mv[:, :, 2], op=OP.add)
                S3 = pool.tile([P, C], F32, name="S3")
                gp.tensor_tensor(S3[:], S2[:], Mv[:, :, 1], op=OP.add)
                T1 = pool.tile([P, C], F32, name="T1")
                gp.tensor_tensor(T1[:], Mv[:, :, 1], S3[:], op=OP.mult)
                CALC = pool.tile([P, C], F32, name="CALC")
                gp.tensor_tensor(CALC[:], Mv[:, :, 1], T1[:], op=OP.mult)

                # sgn = clamp(calc, -1, 1); e0 = calc==1; e1 = calc==0
                SGA = pool.tile([P, C], F32, name="SGA")
                dve.tensor_scalar_max(SGA[:], CALC[:], -1.0)
                SGN = pool.tile([P, C], F32, name="SGN")
                dve.tensor_scalar_min(SGN[:], SGA[:], 1.0)
                E0 = pool.tile([P, C], U8, name="E0")
                dve.tensor_scalar(E0[:], CALC[:], 1.0, None, op0=OP.is_equal)
                E1 = pool.tile([P, C], U8, name="E1")
                dve.tensor_scalar(E1[:], CALC[:], 0.0, None, op0=OP.is_equal)

                # --- masks: MASK[g] = (M[g] == sgn)
                MASK = pool.tile([P, 3 * C], F32, name="MASK", bufs=1)
                MASKv = MASK[:].rearrange("p (c g) -> p c g", g=3)
                sgnb = SGN[:].broadcast_to((P, C, 3))
                dve.tensor_tensor(MASKv, Mv, sgnb, op=OP.is_equal)

                # --- masked groups (j-indexed planes, g packed inner)
                GM = []
                for j in range(3):
                    gm = pool.tile([P, 3 * C], F32, name=f"GM{j}")
                    gmv = gm[:].rearrange("p (c g) -> p c g", g=3)
                    eng = gp if j == 2 else dve
                    eng.tensor_tensor(gmv, V[j], MASKv, op=OP.mult)
                    GM.append(gm)
                GMv = [gm[:].rearrange("p (c g) -> p c g", g=3) for gm in GM]

                # --- probe: PR[g] = GM_idx[g]; idx: 2 default, 1 if calc==0, 0 if calc==1
                PR = pool.tile([P, 3 * C], F32, name="PR")
                PRv = PR[:].rearrange("p (c g) -> p c g", g=3)
                dve.tensor_copy(PR[:], GM[2][:])
                e1b = E1[:].broadcast_to((P, C, 3))
                e0b = E0[:].broadcast_to((P, C, 3))
                _copy_pred(dve, PRv, e1b, GMv[1])
                _copy_pred(dve, PRv, e0b, GMv[0])

                # --- choice: first-argmax of (pu, pn, pd)
                CN = pool.tile([P, C], U8, name="CN")
                dve.tensor_tensor(CN[:], PRv[:, :, 1], PRv[:, :, 2], op=OP.is_ge)
                MND = pool.tile([P, C], F32, name="MND")
                dve.tensor_tensor(MND[:], PRv[:, :, 1], PRv[:, :, 2], op=OP.max)
                CU = pool.tile([P, C], U8, name="CU")
                dve.tensor_tensor(CU[:], PRv[:, :, 0], MND[:], op=OP.is_ge)

                # --- output select: O[:, j] = GM_j[:, choice]
                OT = pool.tile([P, 3 * C], F32, name="OT")
                Ov = OT[:].rearrange("p (c j) -> p c j", j=3)
                for j in range(3):
                    ocol = Ov[:, :, j]
                    if use_act:
                        nc.scalar.copy(ocol, GMv[j][:, :, 2])
                    else:
                        dve.tensor_copy(ocol, GMv[j][:, :, 2])
                    _copy_pred(dve, ocol, CN[:], GMv[j][:, :, 1])
                    _copy_pred(dve, ocol, CU[:], GMv[j][:, :, 0])

                nc.sync.dma_start(ot[t], OT[:])

    return nc


# --------------------------------------------------------------------------
# Entry point
# --------------------------------------------------------------------------
_BUILT = {}


def _get_nc(R: int, C: int):
    key = (R, C)
    if key not in _BUILT:
        nc = build_nc(
            R, C, use_gpsimd=os.environ.get("USE_GP", "1") == "1"
        )
        split_multi_waits(nc)
        _BUILT[key] = nc
    return _BUILT[key]


KERNEL_C = int(os.environ.get("KERNEL_C", "512"))


def kernel(inputs) -> np.ndarray:
    x = np.ascontiguousarray(np.asarray(inputs, dtype=np.float32))
    n = x.shape[0]
    assert n % N_CORES == 0
    r = n // N_CORES
    nc = _get_nc(r, KERNEL_C)
    shards = x.reshape(N_CORES, r, 9)
    from concourse.bass_utils import run_bass_kernel_spmd

    res = run_bass_kernel_spmd(
        nc, [{"x": shards[i]} for i in range(N_CORES)], core_ids=list(range(N_CORES))
    )
    return np.concatenate([r_["o"] for r_ in res.results], axis=0)
